# revision 14
# baseline (speedup 1.0000x reference)
"""Trainium2 Bass kernel: 16-head attention (B=2, S=2048, D=1024), 8-core SPMD.

Sharding: core c -> batch b = c//4, head-group g = c%4 (heads 4g..4g+3).
Each core computes its 4 heads' attention weights [4,2048,2048] plus a
partial output projection [2048,1024]; host sums partials over the 4
head-groups of each batch and adds the (bv @ Wo + bo) constant row.

Device pipeline per core (all matmuls float32r, PSUM accumulate fp32):
  1. QKV projections from host-transposed inputs xT [1024, 2048]:
     qT,kT [256,2048] (head-dim on partitions), v [2048,256] (seq on
     partitions, ones column appended per (k-chunk, head) for row sums).
  2. Per head: scores in BOTH orientations on PE (recompute beats
     transposing): natural [q,k] -> exp (ACT, accum_out row sums) ->
     normalize (DVE tensor_scalar per-partition 1/sum) -> HBM weights;
     transposed [k,q] -> exp -> context matmul (v-stationary, PSUM
     accumulate over k) with sums row from the ones column -> normalize
     via PE outer-product broadcast + DVE multiply.
  3. Output projection (context^T stationary, Wo moving) -> partial.
"""

import sys

sys.path.insert(0, "/opt/trn_rl_repo")

import numpy as np

from concourse import bacc, mybir
from concourse.bass_utils import run_bass_kernel_spmd
from concourse.tile import TileContext

F32 = mybir.dt.float32
F32R = mybir.dt.float32r
EXP = mybir.ActivationFunctionType.Exp

S = 2048      # sequence length
C = 1024      # d_model
DH = 256      # head-group width (4 heads x 64)
NH = 4        # heads per core
DK = 64       # head dim
NCH = C // 128   # 8 contraction chunks
NQT = S // 128   # 16 q/k tiles
B_FULL = 2
H_FULL = 16
N_CORES = 8

# DVE-staged f32r: DMA into f32 tiles then copy-cast, if direct f32r DMA
# is rejected by the BIR verifier. Toggled by _build on failure.
MM_DT = F32R


def _build_program(mm_dt=MM_DT):
    nc = bacc.Bacc(None, target_bir_lowering=False)

    xq = nc.dram_tensor("xq", [C, S], mm_dt, kind="ExternalInput")
    xk = nc.dram_tensor("xk", [C, S], mm_dt, kind="ExternalInput")
    xv = nc.dram_tensor("xv", [C, S], mm_dt, kind="ExternalInput")
    wq = nc.dram_tensor("wq", [C, DH], mm_dt, kind="ExternalInput")
    wk = nc.dram_tensor("wk", [C, DH], mm_dt, kind="ExternalInput")
    wv = nc.dram_tensor("wv", [C, DH], mm_dt, kind="ExternalInput")
    wo = nc.dram_tensor("wo", [DH, C], mm_dt, kind="ExternalInput")
    bq = nc.dram_tensor("bq", [128, 2], F32, kind="ExternalInput")
    bk = nc.dram_tensor("bk", [128, 2], F32, kind="ExternalInput")
    attnw = nc.dram_tensor("attnw", [NH * S, S], F32, kind="ExternalOutput")
    partial = nc.dram_tensor("partial", [S, C], F32, kind="ExternalOutput")

    with TileContext(nc) as tc:
        with tc.tile_pool(name="pers", bufs=1) as pers:
            # persistent SBUF: projections, wo, context accumulator
            qT = pers.tile([128, 2 * S], mm_dt, tag="qT")   # [d-tile 2][128, 2048]
            kT = pers.tile([128, 2 * S], mm_dt, tag="kT")
            # v augmented: per k-chunk sc, per head h: 64 v cols + ones col
            va = pers.tile([128, NQT * (NH * 65)], mm_dt, tag="va")
            wo_sb = pers.tile([128, 2 * C], mm_dt, tag="wo")
            ctxT = pers.tile([128, 2 * S], mm_dt, tag="ctxT")
            bq_sb = pers.tile([128, 2], F32, tag="bq")
            bk_sb = pers.tile([128, 2], F32, tag="bk")

            nc.sync.dma_start(bq_sb[:, :], bq[:, :])
            nc.sync.dma_start(bk_sb[:, :], bk[:, :])
            for ch in range(2):
                nc.sync.dma_start(wo_sb[:, ch * C:(ch + 1) * C],
                                  wo[ch * 128:(ch + 1) * 128, :])

            # ---------------- Phase 1: projections ----------------
            with tc.tile_pool(name="ph1w", bufs=1) as ph1w, \
                 tc.tile_pool(name="ph1xv", bufs=1) as ph1xv, \
                 tc.tile_pool(name="xs", bufs=2) as pxs, \
                 tc.tile_pool(name="PP", bufs=2, space="PSUM") as PP:
                wq_sb = ph1w.tile([128, NCH * DH], mm_dt, tag="wq")
                wk_sb = ph1w.tile([128, NCH * DH], mm_dt, tag="wk")
                wv_sb = ph1w.tile([128, NCH * DH], mm_dt, tag="wv")
                for ch in range(NCH):
                    sl = slice(ch * DH, (ch + 1) * DH)
                    rs = slice(ch * 128, (ch + 1) * 128)
                    nc.sync.dma_start(wq_sb[:, sl], wq[rs, :])
                    nc.sync.dma_start(wk_sb[:, sl], wk[rs, :])
                    nc.sync.dma_start(wv_sb[:, sl], wv[rs, :])

                # q and k projections: out qT/kT [2 d-tiles][128, 2048]
                for name, xdram, wsb, bsb, outT in (
                    ("q", xq, wq_sb, bq_sb, qT),
                    ("k", xk, wk_sb, bk_sb, kT),
                ):
                    ps = [PP.tile([128, S], F32, tag="PP",
                                  name=f"ps_{name}{i}") for i in range(2)]
                    for ch in range(NCH):
                        xt = pxs.tile([128, S], mm_dt, tag="xs")
                        nc.sync.dma_start(xt[:, :],
                                          xdram[ch * 128:(ch + 1) * 128, :])
                        for dt_i in range(2):
                            lhs = wsb[:, ch * DH + dt_i * 128:
                                      ch * DH + (dt_i + 1) * 128]
                            for j in range(4):
                                nc.tensor.matmul(
                                    ps[dt_i][:, j * 512:(j + 1) * 512],
                                    lhs, xt[:, j * 512:(j + 1) * 512],
                                    start=(ch == 0), stop=(ch == NCH - 1))
                    for dt_i in range(2):
                        nc.vector.tensor_scalar_add(
                            outT[:, dt_i * S:(dt_i + 1) * S],
                            ps[dt_i][:, :], bsb[:, dt_i:dt_i + 1])

                # v projection: seq on partitions; ones column via memset
                xv_sb = ph1xv.tile([128, NCH * S], mm_dt, tag="xv")
                for ch in range(NCH):
                    nc.sync.dma_start(xv_sb[:, ch * S:(ch + 1) * S],
                                      xv[ch * 128:(ch + 1) * 128, :])
                # ones columns of va (col 64 of each 65-wide group) via a
                # strided copy from an f32 ones tile (memset can't write f32r)
                ones_f = ph1w.tile([128, DK], F32, tag="onesf")
                nc.vector.memset(ones_f[:, :], 1.0)
                va_view = va[:, :].rearrange("p (g m) -> p g m", m=65)
                nc.vector.tensor_copy(
                    va_view[:, :, 64:65],
                    ones_f[:, :].rearrange("p (g x) -> p g x", x=1))
                for sc in range(NQT):
                    psv = PP.tile([128, DH], F32, tag="PP")
                    for ch in range(NCH):
                        nc.tensor.matmul(
                            psv[:, :],
                            xv_sb[:, ch * S + sc * 128:ch * S + (sc + 1) * 128],
                            wv_sb[:, ch * DH:(ch + 1) * DH],
                            start=(ch == 0), stop=(ch == NCH - 1))
                    base = sc * (NH * 65)
                    for h in range(NH):
                        nc.vector.tensor_copy(
                            va[:, base + h * 65:base + h * 65 + 64],
                            psv[:, h * DK:(h + 1) * DK])

            # ---------------- Phase 2: heads ----------------
            with tc.tile_pool(name="Sn", bufs=2, space="PSUM") as Sn, \
                 tc.tile_pool(name="St", bufs=2, space="PSUM") as St, \
                 tc.tile_pool(name="ctxp", bufs=2, space="PSUM") as ctxp, \
                 tc.tile_pool(name="pU", bufs=8) as pU, \
                 tc.tile_pool(name="pW", bufs=6) as pW, \
                 tc.tile_pool(name="pUT", bufs=4) as pUT, \
                 tc.tile_pool(name="sm", bufs=6) as sm:
                ones1f = sm.tile([1, DK], F32, tag="ones1f")
                nc.vector.memset(ones1f[:, :], 1.0)
                ones1 = sm.tile([1, DK], mm_dt, tag="ones1")
                nc.vector.tensor_copy(ones1[:, :], ones1f[:, :])

                # heads processed in pairs (even head in PE rows 0:64, odd
                # head in rows 64:128) so the two K=64 matmuls run
                # concurrently in different row-groups of the array
                for hp in range(2):
                    h_e, h_o = 2 * hp, 2 * hp + 1
                    dt_i = hp
                    qTe = qT[0:DK, dt_i * S:(dt_i + 1) * S]
                    kTe = kT[0:DK, dt_i * S:(dt_i + 1) * S]
                    qTo = qT[DK:128, dt_i * S:(dt_i + 1) * S]
                    kTo = kT[DK:128, dt_i * S:(dt_i + 1) * S]

                    # --- natural: softmax weights out ---
                    for qt in range(NQT):
                        Us = {}
                        sums2 = {p: sm.tile([128, 2], F32, tag=f"sums2{p}",
                                            name=f"sums2{p}")
                                 for p in (0, 1)}
                        for kh in range(2):
                            pse = Sn.tile([128, 1024], F32, tag="Sn",
                                          name="pse")
                            pso = Sn.tile([128, 1024], F32, tag="Sn",
                                          name="pso")
                            for j in range(2):
                                off = kh * 1024 + j * 512
                                nc.tensor.matmul(
                                    pse[:, j * 512:(j + 1) * 512],
                                    qTe[:, qt * 128:(qt + 1) * 128],
                                    kTe[:, off:off + 512],
                                    start=True, stop=True)
                                nc.tensor.matmul(
                                    pso[:, j * 512:(j + 1) * 512],
                                    qTo[:, qt * 128:(qt + 1) * 128],
                                    kTo[:, off:off + 512],
                                    start=True, stop=True)
                            for p, psb in ((0, pse), (1, pso)):
                                U = pU.tile([128, 1024], F32, tag="U",
                                            name=f"U{p}")
                                nc.scalar.activation(
                                    U[:, :], psb[:, :], EXP,
                                    accum_out=sums2[p][:, kh:kh + 1])
                                Us[(p, kh)] = U
                        for p, h in ((0, h_e), (1, h_o)):
                            sumt = sm.tile([128, 1], F32, tag="sumt",
                                           name="sumt")
                            nc.vector.tensor_add(sumt[:, :],
                                                 sums2[p][:, 0:1],
                                                 sums2[p][:, 1:2])
                            rec = sm.tile([128, 1], F32, tag="rec",
                                          name="rec")
                            nc.vector.reciprocal(rec[:, :], sumt[:, :])
                            for kh in range(2):
                                W = pW.tile([128, 1024], F32, tag="W",
                                            name="W")
                                nc.vector.tensor_scalar_mul(
                                    W[:, :], Us[(p, kh)][:, :], rec[:, :])
                                nc.sync.dma_start(
                                    attnw[h * S + qt * 128:
                                          h * S + (qt + 1) * 128,
                                          kh * 1024:(kh + 1) * 1024],
                                    W[:, :])

                    # --- transposed scores + context (q in quarters) ---
                    for qq in range(4):
                        ctxe = ctxp.tile([65, 512], F32, tag="ctx",
                                         name="ctxe")
                        ctxo = ctxp.tile([65, 512], F32, tag="ctx",
                                         name="ctxo")
                        for kt in range(NQT):
                            pTe = St.tile([128, 512], F32, tag="St",
                                          name="pTe")
                            pTo = St.tile([128, 512], F32, tag="St",
                                          name="pTo")
                            nc.tensor.matmul(
                                pTe[:, :], kTe[:, kt * 128:(kt + 1) * 128],
                                qTe[:, qq * 512:(qq + 1) * 512],
                                start=True, stop=True)
                            nc.tensor.matmul(
                                pTo[:, :], kTo[:, kt * 128:(kt + 1) * 128],
                                qTo[:, qq * 512:(qq + 1) * 512],
                                start=True, stop=True)
                            UTe = pUT.tile([128, 512], mm_dt, tag="UT",
                                           name="UTe")
                            UTo = pUT.tile([128, 512], mm_dt, tag="UT",
                                           name="UTo")
                            nc.scalar.activation(UTe[:, :], pTe[:, :], EXP)
                            nc.scalar.activation(UTo[:, :], pTo[:, :], EXP)
                            base = kt * (NH * 65)
                            nc.tensor.matmul(
                                ctxe[:, :],
                                va[:, base + h_e * 65:base + (h_e + 1) * 65],
                                UTe[:, :],
                                start=(kt == 0), stop=(kt == NQT - 1))
                            nc.tensor.matmul(
                                ctxo[:, :],
                                va[:, base + h_o * 65:base + (h_o + 1) * 65],
                                UTo[:, :],
                                start=(kt == 0), stop=(kt == NQT - 1))
                        for p, ctx in ((0, ctxe), (1, ctxo)):
                            pb = DK * p
                            # normalize: ctxN = ctx[0:64] * (1/sums_row)
                            rrow = sm.tile([1, 512], F32, tag="rrow",
                                           name="rrow")
                            nc.vector.reciprocal(rrow[:, :], ctx[64:65, :])
                            rrow_r = sm.tile([1, 512], mm_dt, tag="rrowr",
                                             name="rrow_r")
                            nc.vector.tensor_copy(rrow_r[:, :], rrow[:, :])
                            B_ps = Sn.tile([64, 512], F32, tag="Sn",
                                           name="B_ps")
                            nc.tensor.matmul(B_ps[:, :], ones1[:, :],
                                             rrow_r[:, :],
                                             start=True, stop=True)
                            B_sb = sm.tile([64, 512], F32, tag="Bsb",
                                           name="B_sb")
                            nc.vector.tensor_copy(B_sb[:, :], B_ps[:, :])
                            nc.vector.tensor_mul(
                                ctxT[pb:pb + DK,
                                     dt_i * S + qq * 512:
                                     dt_i * S + (qq + 1) * 512],
                                ctx[0:DK, :], B_sb[:, :])

                # ---------------- Phase 3: output projection ----------------
                for qt in range(NQT):
                    po = Sn.tile([128, 1024], F32, tag="Sn")
                    for ch in range(2):
                        lhs = ctxT[:, ch * S + qt * 128:ch * S + (qt + 1) * 128]
                        for j in range(2):
                            nc.tensor.matmul(
                                po[:, j * 512:(j + 1) * 512],
                                lhs, wo_sb[:, ch * C + j * 512:
                                           ch * C + (j + 1) * 512],
                                start=(ch == 0), stop=(ch == 1))
                    ot = pW.tile([128, 1024], F32, tag="W")
                    nc.vector.tensor_copy(ot[:, :], po[:, :])
                    nc.sync.dma_start(partial[qt * 128:(qt + 1) * 128, :],
                                      ot[:, :])

    nc.finalize()
    return nc


_PROGRAM_CACHE = {}


def _get_program():
    if "nc" not in _PROGRAM_CACHE:
        _PROGRAM_CACHE["nc"] = _build_program()
    return _PROGRAM_CACHE["nc"]


def _host_prep(query, key, value, Wq, bq, Wk, bk, Wv, bv, Wo, bo):
    """Build the per-core input maps. Fold 1/sqrt(d_k)=0.125 into Wq/bq."""
    in_maps = []
    q8 = (Wq * np.float32(0.125)).astype(np.float32)
    bq8 = (bq * np.float32(0.125)).astype(np.float32)
    for c in range(N_CORES):
        b, g = divmod(c, NH)
        sl = slice(g * DH, (g + 1) * DH)
        m = {
            "xq": np.ascontiguousarray(query[b].T).astype(np.float32),
            "xk": np.ascontiguousarray(key[b].T).astype(np.float32),
            "xv": np.ascontiguousarray(value[b].T).astype(np.float32),
            "wq": np.ascontiguousarray(q8[:, sl]),
            "wk": np.ascontiguousarray(Wk[:, sl]).astype(np.float32),
            "wv": np.ascontiguousarray(Wv[:, sl]).astype(np.float32),
            "wo": np.ascontiguousarray(Wo[sl, :]).astype(np.float32),
            "bq": np.ascontiguousarray(
                bq8[sl].reshape(2, 128).T).astype(np.float32),
            "bk": np.ascontiguousarray(
                np.asarray(bk)[sl].reshape(2, 128).T).astype(np.float32),
        }
        in_maps.append(m)
    return in_maps


def run(inputs, trace=False):
    """Run on 8 cores; returns ((output, attention_weights), BassKernelResults)."""
    inputs = {k: np.asarray(v) for k, v in inputs.items()}
    nc = _get_program()
    in_maps = _host_prep(**inputs)
    res = run_bass_kernel_spmd(nc, in_maps, list(range(N_CORES)), trace=trace)

    attn = np.empty((B_FULL, H_FULL, S, S), np.float32)
    out = np.zeros((B_FULL, S, C), np.float32)
    for c in range(N_CORES):
        b, g = divmod(c, NH)
        attn[b, g * NH:(g + 1) * NH] = \
            res.results[c]["attnw"].reshape(NH, S, S)
        out[b] += res.results[c]["partial"]
    const_row = (np.asarray(inputs["bv"], np.float32)
                 @ np.asarray(inputs["Wo"], np.float32)
                 + np.asarray(inputs["bo"], np.float32))
    out += const_row[None, None, :]
    return (out, attn), res


def kernel(**inputs):
    (out, attn), _ = run(inputs, trace=False)
    return (out, attn)


# revision 20
# speedup vs baseline: 1.3689x; 1.3689x over previous
"""Trainium2 Bass kernel: 16-head attention (B=2, S=2048, D=1024), 8-core SPMD.

Sharding: core c -> batch b = c//4, head-group g = c%4 (heads 4g..4g+3).
Each core computes its 4 heads' attention weights [4,2048,2048] plus a
partial output projection [2048,1024]; host sums partials over the 4
head-groups of each batch and adds the (bv @ Wo + bo) constant row.

Device pipeline per core (all matmuls float32r, PSUM accumulate fp32):
  1. QKV projections from host-transposed inputs xT [1024, 2048]:
     qT,kT [256,2048] (head-dim on partitions), v [2048,256] (seq on
     partitions, ones column appended per (k-chunk, head) for row sums).
  2. Per head: scores in BOTH orientations on PE (recompute beats
     transposing): natural [q,k] -> exp (ACT, accum_out row sums) ->
     normalize (DVE tensor_scalar per-partition 1/sum) -> HBM weights;
     transposed [k,q] -> exp -> context matmul (v-stationary, PSUM
     accumulate over k) with sums row from the ones column -> normalize
     via PE outer-product broadcast + DVE multiply.
  3. Output projection (context^T stationary, Wo moving) -> partial.
"""

import sys

sys.path.insert(0, "/opt/trn_rl_repo")

import numpy as np

from concourse import bacc, mybir
from concourse.bass_utils import run_bass_kernel_spmd
from concourse.tile import TileContext

F32 = mybir.dt.float32
F32R = mybir.dt.float32r
EXP = mybir.ActivationFunctionType.Exp

S = 2048      # sequence length
C = 1024      # d_model
DH = 256      # head-group width (4 heads x 64)
NH = 4        # heads per core
DK = 64       # head dim
NCH = C // 128   # 8 contraction chunks
NQT = S // 128   # 16 q/k tiles
B_FULL = 2
H_FULL = 16
N_CORES = 8

# DVE-staged f32r: DMA into f32 tiles then copy-cast, if direct f32r DMA
# is rejected by the BIR verifier. Toggled by _build on failure.
MM_DT = F32R


def _build_program(mm_dt=MM_DT):
    nc = bacc.Bacc(None, target_bir_lowering=False)

    xq = nc.dram_tensor("xq", [C, S], mm_dt, kind="ExternalInput")
    xk = nc.dram_tensor("xk", [C, S], mm_dt, kind="ExternalInput")
    xv = nc.dram_tensor("xv", [C, S], mm_dt, kind="ExternalInput")
    wq = nc.dram_tensor("wq", [C, DH], mm_dt, kind="ExternalInput")
    wk = nc.dram_tensor("wk", [C, DH], mm_dt, kind="ExternalInput")
    wv = nc.dram_tensor("wv", [C, DH], mm_dt, kind="ExternalInput")
    wo = nc.dram_tensor("wo", [DH, C], mm_dt, kind="ExternalInput")
    bq = nc.dram_tensor("bq", [128, 2], F32, kind="ExternalInput")
    bk = nc.dram_tensor("bk", [128, 2], F32, kind="ExternalInput")
    attnw = nc.dram_tensor("attnw", [NH * S, S], F32, kind="ExternalOutput")
    partial = nc.dram_tensor("partial", [S, C], F32, kind="ExternalOutput")

    with TileContext(nc) as tc:
        with tc.tile_pool(name="pers", bufs=1) as pers:
            # persistent SBUF: projections, wo, context accumulator
            qT = pers.tile([128, 2 * S], mm_dt, tag="qT")   # [d-tile 2][128, 2048]
            kT = pers.tile([128, 2 * S], mm_dt, tag="kT")
            # v augmented: per k-chunk sc, per head h: 64 v cols + ones col
            va = pers.tile([128, NQT * (NH * 65)], mm_dt, tag="va")
            wo_sb = pers.tile([128, 2 * C], mm_dt, tag="wo")
            ctxT = pers.tile([128, 2 * S], mm_dt, tag="ctxT")
            bq_sb = pers.tile([128, 2], F32, tag="bq")
            bk_sb = pers.tile([128, 2], F32, tag="bk")

            nc.sync.dma_start(bq_sb[:, :], bq[:, :])
            nc.sync.dma_start(bk_sb[:, :], bk[:, :])
            for ch in range(2):
                nc.sync.dma_start(wo_sb[:, ch * C:(ch + 1) * C],
                                  wo[ch * 128:(ch + 1) * 128, :])

            # ---------------- Phase 1: projections ----------------
            with tc.tile_pool(name="ph1w", bufs=1) as ph1w, \
                 tc.tile_pool(name="ph1xv", bufs=1) as ph1xv, \
                 tc.tile_pool(name="xs", bufs=2) as pxs, \
                 tc.tile_pool(name="PP", bufs=2, space="PSUM") as PP:
                wq_sb = ph1w.tile([128, NCH * DH], mm_dt, tag="wq")
                wk_sb = ph1w.tile([128, NCH * DH], mm_dt, tag="wk")
                wv_sb = ph1w.tile([128, NCH * DH], mm_dt, tag="wv")
                for ch in range(NCH):
                    sl = slice(ch * DH, (ch + 1) * DH)
                    rs = slice(ch * 128, (ch + 1) * 128)
                    nc.sync.dma_start(wq_sb[:, sl], wq[rs, :])
                    nc.sync.dma_start(wk_sb[:, sl], wk[rs, :])
                    nc.sync.dma_start(wv_sb[:, sl], wv[rs, :])

                # q and k projections: out qT/kT [2 d-tiles][128, 2048]
                for name, xdram, wsb, bsb, outT in (
                    ("q", xq, wq_sb, bq_sb, qT),
                    ("k", xk, wk_sb, bk_sb, kT),
                ):
                    ps = [PP.tile([128, S], F32, tag="PP",
                                  name=f"ps_{name}{i}") for i in range(2)]
                    for ch in range(NCH):
                        xt = pxs.tile([128, S], mm_dt, tag="xs")
                        nc.sync.dma_start(xt[:, :],
                                          xdram[ch * 128:(ch + 1) * 128, :])
                        for dt_i in range(2):
                            lhs = wsb[:, ch * DH + dt_i * 128:
                                      ch * DH + (dt_i + 1) * 128]
                            for j in range(4):
                                nc.tensor.matmul(
                                    ps[dt_i][:, j * 512:(j + 1) * 512],
                                    lhs, xt[:, j * 512:(j + 1) * 512],
                                    start=(ch == 0), stop=(ch == NCH - 1))
                    for dt_i in range(2):
                        nc.vector.tensor_scalar_add(
                            outT[:, dt_i * S:(dt_i + 1) * S],
                            ps[dt_i][:, :], bsb[:, dt_i:dt_i + 1])

                # v projection: seq on partitions; ones column via memset
                xv_sb = ph1xv.tile([128, NCH * S], mm_dt, tag="xv")
                for ch in range(NCH):
                    nc.sync.dma_start(xv_sb[:, ch * S:(ch + 1) * S],
                                      xv[ch * 128:(ch + 1) * 128, :])
                # ones columns of va (col 64 of each 65-wide group) via a
                # strided copy from an f32 ones tile (memset can't write f32r)
                ones_f = ph1w.tile([128, DK], F32, tag="onesf")
                nc.vector.memset(ones_f[:, :], 1.0)
                va_view = va[:, :].rearrange("p (g m) -> p g m", m=65)
                nc.vector.tensor_copy(
                    va_view[:, :, 64:65],
                    ones_f[:, :].rearrange("p (g x) -> p g x", x=1))
                for sc in range(NQT):
                    psv = PP.tile([128, DH], F32, tag="PP")
                    for ch in range(NCH):
                        nc.tensor.matmul(
                            psv[:, :],
                            xv_sb[:, ch * S + sc * 128:ch * S + (sc + 1) * 128],
                            wv_sb[:, ch * DH:(ch + 1) * DH],
                            start=(ch == 0), stop=(ch == NCH - 1))
                    base = sc * (NH * 65)
                    for h in range(NH):
                        nc.vector.tensor_copy(
                            va[:, base + h * 65:base + h * 65 + 64],
                            psv[:, h * DK:(h + 1) * DK])

            # ---------------- Phase 2: heads ----------------
            with tc.tile_pool(name="Sn", bufs=1, space="PSUM") as Sn, \
                 tc.tile_pool(name="St", bufs=2, space="PSUM") as St, \
                 tc.tile_pool(name="ctxp", bufs=1, space="PSUM") as ctxp, \
                 tc.tile_pool(name="pU", bufs=6) as pU, \
                 tc.tile_pool(name="pW", bufs=6) as pW, \
                 tc.tile_pool(name="pUT", bufs=4) as pUT, \
                 tc.tile_pool(name="sm", bufs=6) as sm, \
                 tc.tile_pool(name="smb", bufs=2) as smb:
                ones1f = sm.tile([1, DK], F32, tag="ones1f")
                nc.vector.memset(ones1f[:, :], 1.0)
                ones1 = sm.tile([1, DK], mm_dt, tag="ones1")
                nc.vector.tensor_copy(ones1[:, :], ones1f[:, :])

                # Zero-padded K=128 stationary buffers: scores stationaries
                # are [64,128] per head; padding to full 128 contraction rows
                # (other head's rows multiplied by zero) keeps the PE array
                # fully occupied so the HAM clock-gate sees it as busy.
                zf = sm.tile([128, 128], F32, tag="zf")
                nc.vector.memset(zf[:, :], 0.0)
                NROT = 4
                NROTK = NQT
                zbufs = {}
                for nm, nrot in (("zqe", NROT), ("zqo", NROT),
                                 ("zke", NROTK), ("zko", NROTK)):
                    zb = pers.tile([128, nrot * 128], mm_dt, tag=nm, name=nm)
                    for s in range(nrot):
                        nc.vector.tensor_copy(
                            zb[:, s * 128:(s + 1) * 128], zf[:, :])
                    zbufs[nm] = zb

                def nat_unit(h, qt, zq, qTh, kTfull):
                    sl = (qt % NROT) * 128
                    pb = DK * (h % 2)
                    zsl = zq[:, sl:sl + 128]
                    nc.vector.tensor_copy(
                        zsl[pb:pb + DK, :], qTh[:, qt * 128:(qt + 1) * 128])
                    Us = []
                    sums2 = sm.tile([128, 2], F32, tag="sums2", name="sums2")
                    for kh in range(2):
                        ps = Sn.tile([128, 1024], F32, tag="Sn", name="ps")
                        for j in range(2):
                            off = kh * 1024 + j * 512
                            nc.tensor.matmul(
                                ps[:, j * 512:(j + 1) * 512],
                                zsl, kTfull[:, off:off + 512],
                                start=True, stop=True)
                        U = pU.tile([128, 1024], F32, tag="U", name="U")
                        nc.scalar.activation(U[:, :], ps[:, :], EXP,
                                             accum_out=sums2[:, kh:kh + 1])
                        Us.append(U)
                    sumt = sm.tile([128, 1], F32, tag="sumt", name="sumt")
                    nc.vector.tensor_add(sumt[:, :], sums2[:, 0:1],
                                         sums2[:, 1:2])
                    rec = sm.tile([128, 1], F32, tag="rec", name="rec")
                    scr = sm.tile([128, 1], F32, tag="scr", name="scr")
                    nc.vector.reciprocal_approx_accurate(rec[:, :], sumt[:, :],
                                                         scr[:, :])
                    for kh in range(2):
                        W = pW.tile([128, 1024], F32, tag="W", name="W")
                        nc.vector.tensor_scalar_mul(W[:, :], Us[kh][:, :],
                                                    rec[:, :])
                        nc.sync.dma_start(
                            attnw[h * S + qt * 128:h * S + (qt + 1) * 128,
                                  kh * 1024:(kh + 1) * 1024],
                            W[:, :])

                for h in range(NH):
                    dt_i = h // 2
                    p = h % 2
                    pb = DK * p
                    qTh = qT[pb:pb + DK, dt_i * S:(dt_i + 1) * S]
                    kTh = kT[pb:pb + DK, dt_i * S:(dt_i + 1) * S]
                    kTfull = kT[0:128, dt_i * S:(dt_i + 1) * S]
                    qTfull = qT[0:128, dt_i * S:(dt_i + 1) * S]
                    zq = zbufs["zqe" if p == 0 else "zqo"]
                    zk = zbufs["zke" if p == 0 else "zko"]

                    for kt in range(NQT):
                        nc.vector.tensor_copy(
                            zk[pb:pb + DK, kt * 128:(kt + 1) * 128],
                            kTh[:, kt * 128:(kt + 1) * 128])

                    # transposed scores + context in q-halves, with natural
                    # units interleaved so PE always has matmul work queued
                    for qh in range(2):
                        ctx = ctxp.tile([65, 1024], F32, tag="ctx",
                                        name="ctx")
                        for kt in range(NQT):
                            zsl = zk[:, kt * 128:(kt + 1) * 128]
                            psT = St.tile([128, 1024], F32, tag="St",
                                          name="psT")
                            for j in range(2):
                                off = qh * 1024 + j * 512
                                nc.tensor.matmul(
                                    psT[:, j * 512:(j + 1) * 512], zsl,
                                    qTfull[:, off:off + 512],
                                    start=True, stop=True)
                            UT = pUT.tile([128, 1024], mm_dt, tag="UT",
                                          name="UT")
                            nc.scalar.activation(UT[:, :], psT[:, :], EXP)
                            base = kt * (NH * 65)
                            for j in range(2):
                                nc.tensor.matmul(
                                    ctx[:, j * 512:(j + 1) * 512],
                                    va[:, base + h * 65:base + (h + 1) * 65],
                                    UT[:, j * 512:(j + 1) * 512],
                                    start=(kt == 0), stop=(kt == NQT - 1))
                            if kt % 2 == 1:
                                nat_unit(h, qh * 8 + kt // 2, zq, qTh,
                                         kTfull)
                        # normalize: ctxN = ctx[0:64] * (1/sums_row)
                        rrow = smb.tile([1, 1024], F32, tag="rrow",
                                       name="rrow")
                        nc.vector.reciprocal(rrow[:, :], ctx[64:65, :])
                        rrow_r = smb.tile([1, 1024], mm_dt, tag="rrowr",
                                         name="rrow_r")
                        nc.vector.tensor_copy(rrow_r[:, :], rrow[:, :])
                        B_ps = Sn.tile([64, 1024], F32, tag="Sn",
                                       name="B_ps")
                        for j in range(2):
                            nc.tensor.matmul(
                                B_ps[:, j * 512:(j + 1) * 512], ones1[:, :],
                                rrow_r[:, j * 512:(j + 1) * 512],
                                start=True, stop=True)
                        B_sb = smb.tile([64, 1024], F32, tag="Bsb",
                                       name="B_sb")
                        nc.vector.tensor_copy(B_sb[:, :], B_ps[:, :])
                        nc.vector.tensor_mul(
                            ctxT[pb:pb + DK,
                                 dt_i * S + qh * 1024:
                                 dt_i * S + (qh + 1) * 1024],
                            ctx[0:DK, :], B_sb[:, :])

                # ---------------- Phase 3: output projection ----------------
                for qt in range(NQT):
                    po = St.tile([128, 1024], F32, tag="St")
                    for ch in range(2):
                        lhs = ctxT[:, ch * S + qt * 128:ch * S + (qt + 1) * 128]
                        for j in range(2):
                            nc.tensor.matmul(
                                po[:, j * 512:(j + 1) * 512],
                                lhs, wo_sb[:, ch * C + j * 512:
                                           ch * C + (j + 1) * 512],
                                start=(ch == 0), stop=(ch == 1))
                    ot = pW.tile([128, 1024], F32, tag="W")
                    nc.vector.tensor_copy(ot[:, :], po[:, :])
                    nc.sync.dma_start(partial[qt * 128:(qt + 1) * 128, :],
                                      ot[:, :])

    nc.finalize()
    return nc


_PROGRAM_CACHE = {}


def _get_program():
    if "nc" not in _PROGRAM_CACHE:
        _PROGRAM_CACHE["nc"] = _build_program()
    return _PROGRAM_CACHE["nc"]


def _host_prep(query, key, value, Wq, bq, Wk, bk, Wv, bv, Wo, bo):
    """Build the per-core input maps. Fold 1/sqrt(d_k)=0.125 into Wq/bq."""
    in_maps = []
    q8 = (Wq * np.float32(0.125)).astype(np.float32)
    bq8 = (bq * np.float32(0.125)).astype(np.float32)
    for c in range(N_CORES):
        b, g = divmod(c, NH)
        sl = slice(g * DH, (g + 1) * DH)
        m = {
            "xq": np.ascontiguousarray(query[b].T).astype(np.float32),
            "xk": np.ascontiguousarray(key[b].T).astype(np.float32),
            "xv": np.ascontiguousarray(value[b].T).astype(np.float32),
            "wq": np.ascontiguousarray(q8[:, sl]),
            "wk": np.ascontiguousarray(Wk[:, sl]).astype(np.float32),
            "wv": np.ascontiguousarray(Wv[:, sl]).astype(np.float32),
            "wo": np.ascontiguousarray(Wo[sl, :]).astype(np.float32),
            "bq": np.ascontiguousarray(
                bq8[sl].reshape(2, 128).T).astype(np.float32),
            "bk": np.ascontiguousarray(
                np.asarray(bk)[sl].reshape(2, 128).T).astype(np.float32),
        }
        in_maps.append(m)
    return in_maps


def run(inputs, trace=False):
    """Run on 8 cores; returns ((output, attention_weights), BassKernelResults)."""
    inputs = {k: np.asarray(v) for k, v in inputs.items()}
    nc = _get_program()
    in_maps = _host_prep(**inputs)
    res = run_bass_kernel_spmd(nc, in_maps, list(range(N_CORES)), trace=trace)

    attn = np.empty((B_FULL, H_FULL, S, S), np.float32)
    out = np.zeros((B_FULL, S, C), np.float32)
    for c in range(N_CORES):
        b, g = divmod(c, NH)
        attn[b, g * NH:(g + 1) * NH] = \
            res.results[c]["attnw"].reshape(NH, S, S)
        out[b] += res.results[c]["partial"]
    const_row = (np.asarray(inputs["bv"], np.float32)
                 @ np.asarray(inputs["Wo"], np.float32)
                 + np.asarray(inputs["bo"], np.float32))
    out += const_row[None, None, :]
    return (out, attn), res


def kernel(**inputs):
    (out, attn), _ = run(inputs, trace=False)
    return (out, attn)
